# revision 3
# baseline (speedup 1.0000x reference)
"""Grouped GEMM (MoE routing) kernel for 8 Trainium2 NeuronCores.

out[off_g : off_g + size_g] = A[off_g : off_g + size_g] @ B[g]   for g in 0..63
A: [524288, 256] f32, B: [64, 256, 256] f32, groups are contiguous row ranges.

Strategy (hardcoded, from the sharding hint "expert-parallel / data-parallel"):
  - Split each group into 2 pieces (tile granularity), sort the 128 pieces by
    tile count, snake-assign one piece per (slot, core): slot i takes pieces
    ranked [8i, 8i+8). Every core runs an IDENTICAL static schedule of
    T = sum(m_i) 128-row tiles (m_i = max tile count in octile i; shorter
    pieces zero-padded), referencing per-slot expert weights resident in SBUF.
  - All device I/O is bf16 (tolerance 2e-2 >> bf16 matmul error ~3e-3):
    host casts A/B to bf16, packs each core's pieces back to back and
    pre-transposes to AT [256, T*128] so the contraction dim is the SBUF
    partition dim; output comes back as bf16 [128, T*N] (tile-row-major,
    fully contiguous per partition) and is upcast + scattered on host.
  - Device: per-core B (R slots, bf16) stays resident in SBUF; A streams in
    W-tile blocks (1 MB per DMA); per 128-row tile: 2 accumulating matmuls
    (K=256 split across two 128-partition chunks) into half of a [128, 512]
    f32 PSUM bank; one dtype-converting copy per tile PAIR moves PSUM->SBUF
    bf16, alternating between the DVE and ACT engines; batched out DMA.
"""

import os

import numpy as np

NCORES = 8
TILE = 128
K = 256
N = 256

# matmul operand dtype on device: "bfloat16" (fast) / "float32r" / "float32".
MM_DTYPE = os.environ.get("BASS_GG_DTYPE", "bfloat16")
W_TILES = int(os.environ.get("BASS_GG_W", "32"))  # tiles per A/out block
SPLIT_Q = int(os.environ.get("BASS_GG_Q", "2"))  # pieces per group

LAST_EXEC_NS = None

_prog_cache = {}


def _schedule(sizes, offsets):
    """-> (cells [nslot][NCORES] of (row_off, nrows, group), m [nslot])."""
    sizes = np.asarray(sizes, dtype=np.int64)
    offsets = np.asarray(offsets, dtype=np.int64)
    pieces = []  # (ntiles, row_off, nrows, group)
    for g in range(len(sizes)):
        sz = int(sizes[g])
        nt = (sz + TILE - 1) // TILE
        base, rem = divmod(nt, SPLIT_Q)
        r0 = 0
        for q in range(SPLIT_Q):
            pt = base + (1 if q < rem else 0)
            if pt == 0:
                continue
            nrows = min(sz - r0, pt * TILE)
            pieces.append((pt, int(offsets[g]) + r0, nrows, g))
            r0 += nrows
    pieces.sort(key=lambda p: -p[0])
    pad = (-len(pieces)) % NCORES
    pieces += [(0, 0, 0, 0)] * pad
    nslot = len(pieces) // NCORES
    cells, m = [], []
    for i in range(nslot):
        octile = pieces[i * NCORES : (i + 1) * NCORES]
        mi = octile[0][0]
        if mi == 0:
            continue
        m.append(mi)
        cells.append([(p[1], p[2], p[3]) for p in octile])
    return cells, m


def _build_program(m_list, dtype_name, w_tiles, repeat):
    import concourse.tile as tile
    from concourse import bacc, mybir

    DT = getattr(mybir.dt, dtype_name)
    ODT = mybir.dt.bfloat16 if dtype_name == "bfloat16" else mybir.dt.float32
    R = len(m_list)
    T = int(sum(m_list))

    nc = bacc.Bacc(
        "TRN2",
        target_bir_lowering=False,
        debug=False,
        enable_asserts=False,
        num_devices=NCORES,
    )
    AT = nc.dram_tensor("AT", [K, T * TILE], DT, kind="ExternalInput").ap()
    BW = nc.dram_tensor("BW", [128, R * 2 * N], DT, kind="ExternalInput").ap()
    OUT = nc.dram_tensor("OUT", [128, T * N], ODT, kind="ExternalOutput").ap()

    slot_of = []
    for i, mi in enumerate(m_list):
        slot_of += [i] * int(mi)

    with tile.TileContext(nc) as tc:
        with tc.tile_pool(name="bpool", bufs=1) as bpool, \
             tc.tile_pool(name="apool", bufs=3) as apool, \
             tc.tile_pool(name="opool", bufs=3) as opool, \
             tc.tile_pool(name="psum", bufs=4, space="PSUM") as pspool:
            b_sb = bpool.tile([128, R * 2 * N], DT)
            nc.sync.dma_start(out=b_sb, in_=BW)
            nblk = (T + w_tiles - 1) // w_tiles
            copy_alt = 0
            for rep in range(repeat):
                for blk in range(nblk):
                    t0 = blk * w_tiles
                    w = min(w_tiles, T - t0)
                    a0 = apool.tile([128, w_tiles * TILE], DT, tag="a0")
                    a1 = apool.tile([128, w_tiles * TILE], DT, tag="a1")
                    nc.sync.dma_start(
                        out=a0[:, : w * TILE],
                        in_=AT[0:128, t0 * TILE : (t0 + w) * TILE],
                    )
                    nc.sync.dma_start(
                        out=a1[:, : w * TILE],
                        in_=AT[128:256, t0 * TILE : (t0 + w) * TILE],
                    )
                    ob = opool.tile([128, w_tiles * N], ODT, tag="ob")
                    for tp in range(0, w, 2):
                        pw = min(2, w - tp)
                        ps = pspool.tile([128, pw * N], mybir.dt.float32)
                        for t in range(tp, tp + pw):
                            s = slot_of[t0 + t]
                            c0 = (t - tp) * N
                            nc.tensor.matmul(
                                ps[:, c0 : c0 + N],
                                lhsT=a0[:, t * TILE : (t + 1) * TILE],
                                rhs=b_sb[:, (2 * s) * N : (2 * s + 1) * N],
                                start=True,
                                stop=False,
                            )
                            nc.tensor.matmul(
                                ps[:, c0 : c0 + N],
                                lhsT=a1[:, t * TILE : (t + 1) * TILE],
                                rhs=b_sb[:, (2 * s + 1) * N : (2 * s + 2) * N],
                                start=False,
                                stop=True,
                            )
                        dst = ob[:, tp * N : (tp + pw) * N]
                        if copy_alt == 0:
                            nc.vector.tensor_copy(out=dst, in_=ps)
                        else:
                            nc.scalar.copy(out=dst, in_=ps)
                        copy_alt ^= 1
                    nc.scalar.dma_start(
                        out=OUT[:, t0 * N : (t0 + w) * N], in_=ob[:, : w * N]
                    )
    nc.compile()
    return nc


def _get_program(m_key, dtype_name, w_tiles, repeat=1):
    key = (m_key, dtype_name, w_tiles, repeat)
    if key not in _prog_cache:
        _prog_cache[key] = _build_program(list(m_key), dtype_name, w_tiles, repeat)
    return _prog_cache[key]


def _np_dtype(dtype_name):
    if dtype_name == "bfloat16":
        from ml_dtypes import bfloat16

        return np.dtype(bfloat16)
    return np.dtype(np.float32)


def _pack_inputs(A, B, cells, m, T):
    """-> in_maps list of {"AT", "BW"} per core (device dtypes)."""
    dt = _np_dtype(MM_DTYPE)
    A16 = np.ascontiguousarray(A).astype(dt)
    B16 = np.ascontiguousarray(B).astype(dt)
    R = len(m)
    starts = np.concatenate([[0], np.cumsum(m)[:-1]]).astype(np.int64)
    in_maps = []
    for c in range(NCORES):
        at = np.zeros((K, T * TILE), dtype=dt)
        bw = np.zeros((128, R, 2, N), dtype=dt)
        for i in range(R):
            row_off, nrows, g = cells[i][c]
            dst = int(starts[i]) * TILE
            if nrows > 0:
                at[:, dst : dst + nrows] = A16[row_off : row_off + nrows].T
            bw[:, i] = B16[g].reshape(2, 128, N).transpose(1, 0, 2)
        in_maps.append({"AT": at, "BW": bw.reshape(128, R * 2 * N)})
    return in_maps


def _unpack_outputs(results, cells, m, T, M):
    starts = np.concatenate([[0], np.cumsum(m)[:-1]]).astype(np.int64)
    out = np.zeros((M, N), dtype=np.float32)
    for c in range(NCORES):
        oc = np.asarray(results[c]["OUT"])
        rows = (
            oc.reshape(128, T, N)
            .transpose(1, 0, 2)
            .reshape(T * TILE, N)
            .astype(np.float32)
        )
        for i in range(len(m)):
            row_off, nrows, _g = cells[i][c]
            src = int(starts[i]) * TILE
            if nrows > 0:
                out[row_off : row_off + nrows] = rows[src : src + nrows]
    return out


def kernel(A, B, batch_sizes, batch_offsets, batch_padded_offsets):
    global LAST_EXEC_NS
    from concourse.bass_utils import run_bass_kernel_spmd

    A = np.asarray(A, dtype=np.float32)
    B = np.asarray(B, dtype=np.float32)
    sizes = np.asarray(batch_sizes, dtype=np.int64)
    offsets = np.asarray(batch_offsets, dtype=np.int64)

    M = A.shape[0]
    cells, m = _schedule(sizes, offsets)
    T = int(sum(m))

    nc = _get_program(tuple(int(x) for x in m), MM_DTYPE, W_TILES)
    in_maps = _pack_inputs(A, B, cells, m, T)

    trace = bool(int(os.environ.get("BASS_GG_TRACE", "0")))
    res = run_bass_kernel_spmd(
        nc,
        in_maps,
        core_ids=list(range(NCORES)),
        trace=trace,
        tmpdir=os.environ.get("BASS_GG_TRACE_DIR") or None,
    )
    LAST_EXEC_NS = res.exec_time_ns

    return _unpack_outputs(res.results, cells, m, T, M)


# revision 5
# speedup vs baseline: 1.2252x; 1.2252x over previous
"""Grouped GEMM (MoE routing) kernel for 8 Trainium2 NeuronCores.

out[off_g : off_g + size_g] = A[off_g : off_g + size_g] @ B[g]   for g in 0..63
A: [524288, 256] f32, B: [64, 256, 256] f32, groups are contiguous row ranges.

Strategy (hardcoded, from the sharding hint "expert-parallel / data-parallel"):
  - Split each group into 2 pieces (tile granularity), sort the 128 pieces by
    tile count, snake-assign one piece per (slot, core): slot i takes pieces
    ranked [8i, 8i+8). Every core runs an IDENTICAL static schedule of
    T = sum(m_i) 128-row tiles (m_i = max tile count in octile i; shorter
    pieces zero-padded), referencing per-slot expert weights resident in SBUF.
  - All device I/O is bf16 (tolerance 2e-2 >> bf16 matmul error ~3e-3):
    host casts A/B to bf16, packs each core's pieces back to back and
    pre-transposes to AT [256, T*128] so the contraction dim is the SBUF
    partition dim; output comes back as bf16 [128, T*N] (tile-row-major,
    fully contiguous per partition) and is upcast + scattered on host.
  - Device: per-core B (R slots, bf16) stays resident in SBUF; A streams in
    W-tile blocks (1 MB per DMA); per 128-row tile: 2 accumulating matmuls
    (K=256 split across two 128-partition chunks) into half of a [128, 512]
    f32 PSUM bank; one dtype-converting copy per tile PAIR moves PSUM->SBUF
    bf16, alternating between the DVE and ACT engines; batched out DMA.
"""

import os

import numpy as np

NCORES = 8
TILE = 128
K = 256
N = 256

# matmul operand dtype on device: "bfloat16" (fast) / "float32r" / "float32".
MM_DTYPE = os.environ.get("BASS_GG_DTYPE", "bfloat16")
W_TILES = int(os.environ.get("BASS_GG_W", "32"))  # tiles per A/out block
PIECE_TILES = int(os.environ.get("BASS_GG_PT", "32"))  # target piece size
OPT_ITERS = int(os.environ.get("BASS_GG_OPT", "15000"))  # schedule hill-climb

LAST_EXEC_NS = None

_prog_cache = {}


def _opt_pieces(ntiles):
    """Cut groups into ~PIECE_TILES pieces, then hill-climb intra-group tile
    transfers to minimize T = sum of per-octile maxima (deterministic seed).
    -> (vals, grp) parallel lists."""
    import random

    vals, grp = [], []
    for g, n in enumerate(ntiles):
        n = int(n)
        if n == 0:
            continue
        k = max(1, round(n / PIECE_TILES))
        base, rem = divmod(n, k)
        for j in range(k):
            vals.append(base + (1 if j < rem else 0))
            grp.append(g)

    def T_of(v):
        s = np.sort(np.asarray(v))[::-1]
        pad = (-len(s)) % NCORES
        if pad:
            s = np.concatenate([s, np.zeros(pad, np.int64)])
        return int(s.reshape(-1, NCORES)[:, 0].sum())

    bygroup = {}
    for i, g in enumerate(grp):
        bygroup.setdefault(g, []).append(i)
    multi = [idx for idx in bygroup.values() if len(idx) > 1]
    if multi and OPT_ITERS > 0:
        rnd = random.Random(0)
        T = T_of(vals)
        for _ in range(OPT_ITERS):
            idx = multi[rnd.randrange(len(multi))]
            i, j = rnd.sample(idx, 2)
            d = rnd.choice((1, 2, 3))
            if vals[i] - d < 1:
                continue
            vals[i] -= d
            vals[j] += d
            T2 = T_of(vals)
            if T2 <= T:
                T = T2
            else:
                vals[i] += d
                vals[j] -= d
    return vals, grp


def _schedule(sizes, offsets):
    """-> (cells [nslot][NCORES] of (row_off, nrows, group), m [nslot])."""
    sizes = np.asarray(sizes, dtype=np.int64)
    offsets = np.asarray(offsets, dtype=np.int64)
    ntiles = (sizes + TILE - 1) // TILE
    vals, grp = _opt_pieces(ntiles)
    # assign row ranges within each group in piece order
    consumed = {g: 0 for g in range(len(sizes))}
    pieces = []  # (ntiles, row_off, nrows, group)
    for v, g in zip(vals, grp):
        r0 = consumed[g]
        nrows = min(int(sizes[g]) - r0, v * TILE)
        consumed[g] = r0 + nrows
        pieces.append((v, int(offsets[g]) + r0, nrows, g))
    pieces.sort(key=lambda p: -p[0])
    pad = (-len(pieces)) % NCORES
    pieces += [(0, 0, 0, 0)] * pad
    nslot = len(pieces) // NCORES
    cells, m = [], []
    for i in range(nslot):
        octile = pieces[i * NCORES : (i + 1) * NCORES]
        mi = octile[0][0]
        if mi == 0:
            continue
        m.append(mi)
        cells.append([(p[1], p[2], p[3]) for p in octile])
    return cells, m


def _build_program(m_list, dtype_name, w_tiles, repeat):
    import concourse.tile as tile
    from concourse import bacc, mybir

    DT = getattr(mybir.dt, dtype_name)
    ODT = mybir.dt.bfloat16 if dtype_name == "bfloat16" else mybir.dt.float32
    R = len(m_list)
    T = int(sum(m_list))

    nc = bacc.Bacc(
        "TRN2",
        target_bir_lowering=False,
        debug=False,
        enable_asserts=False,
        num_devices=NCORES,
    )
    AT = nc.dram_tensor("AT", [K, T * TILE], DT, kind="ExternalInput").ap()
    BW = nc.dram_tensor("BW", [128, R * 2 * N], DT, kind="ExternalInput").ap()
    OUT = nc.dram_tensor("OUT", [128, T * N], ODT, kind="ExternalOutput").ap()

    slot_of = []
    for i, mi in enumerate(m_list):
        slot_of += [i] * int(mi)

    with tile.TileContext(nc) as tc:
        with tc.tile_pool(name="bpool", bufs=1) as bpool, \
             tc.tile_pool(name="apool", bufs=3) as apool, \
             tc.tile_pool(name="opool", bufs=3) as opool, \
             tc.tile_pool(name="psum", bufs=4, space="PSUM") as pspool:
            b_sb = bpool.tile([128, R * 2 * N], DT)
            nc.sync.dma_start(out=b_sb, in_=BW)
            nblk = (T + w_tiles - 1) // w_tiles
            copy_alt = 0
            for rep in range(repeat):
                for blk in range(nblk):
                    t0 = blk * w_tiles
                    w = min(w_tiles, T - t0)
                    a0 = apool.tile([128, w_tiles * TILE], DT, tag="a0")
                    a1 = apool.tile([128, w_tiles * TILE], DT, tag="a1")
                    nc.sync.dma_start(
                        out=a0[:, : w * TILE],
                        in_=AT[0:128, t0 * TILE : (t0 + w) * TILE],
                    )
                    nc.sync.dma_start(
                        out=a1[:, : w * TILE],
                        in_=AT[128:256, t0 * TILE : (t0 + w) * TILE],
                    )
                    ob = opool.tile([128, w_tiles * N], ODT, tag="ob")
                    for tp in range(0, w, 2):
                        pw = min(2, w - tp)
                        ps = pspool.tile([128, pw * N], mybir.dt.float32)
                        for t in range(tp, tp + pw):
                            s = slot_of[t0 + t]
                            c0 = (t - tp) * N
                            nc.tensor.matmul(
                                ps[:, c0 : c0 + N],
                                lhsT=a0[:, t * TILE : (t + 1) * TILE],
                                rhs=b_sb[:, (2 * s) * N : (2 * s + 1) * N],
                                start=True,
                                stop=False,
                            )
                            nc.tensor.matmul(
                                ps[:, c0 : c0 + N],
                                lhsT=a1[:, t * TILE : (t + 1) * TILE],
                                rhs=b_sb[:, (2 * s + 1) * N : (2 * s + 2) * N],
                                start=False,
                                stop=True,
                            )
                        dst = ob[:, tp * N : (tp + pw) * N]
                        if copy_alt == 0:
                            nc.vector.tensor_copy(out=dst, in_=ps)
                        else:
                            nc.scalar.copy(out=dst, in_=ps)
                        copy_alt ^= 1
                    nc.scalar.dma_start(
                        out=OUT[:, t0 * N : (t0 + w) * N], in_=ob[:, : w * N]
                    )
    nc.compile()
    return nc


def _get_program(m_key, dtype_name, w_tiles, repeat=1):
    key = (m_key, dtype_name, w_tiles, repeat)
    if key not in _prog_cache:
        _prog_cache[key] = _build_program(list(m_key), dtype_name, w_tiles, repeat)
    return _prog_cache[key]


def _np_dtype(dtype_name):
    if dtype_name == "bfloat16":
        from ml_dtypes import bfloat16

        return np.dtype(bfloat16)
    return np.dtype(np.float32)


def _pack_inputs(A, B, cells, m, T):
    """-> in_maps list of {"AT", "BW"} per core (device dtypes)."""
    dt = _np_dtype(MM_DTYPE)
    A16 = np.ascontiguousarray(A).astype(dt)
    B16 = np.ascontiguousarray(B).astype(dt)
    R = len(m)
    starts = np.concatenate([[0], np.cumsum(m)[:-1]]).astype(np.int64)
    in_maps = []
    for c in range(NCORES):
        at = np.zeros((K, T * TILE), dtype=dt)
        bw = np.zeros((128, R, 2, N), dtype=dt)
        for i in range(R):
            row_off, nrows, g = cells[i][c]
            dst = int(starts[i]) * TILE
            if nrows > 0:
                at[:, dst : dst + nrows] = A16[row_off : row_off + nrows].T
            bw[:, i] = B16[g].reshape(2, 128, N).transpose(1, 0, 2)
        in_maps.append({"AT": at, "BW": bw.reshape(128, R * 2 * N)})
    return in_maps


def _unpack_outputs(results, cells, m, T, M):
    starts = np.concatenate([[0], np.cumsum(m)[:-1]]).astype(np.int64)
    out = np.zeros((M, N), dtype=np.float32)
    for c in range(NCORES):
        oc = np.asarray(results[c]["OUT"])
        rows = (
            oc.reshape(128, T, N)
            .transpose(1, 0, 2)
            .reshape(T * TILE, N)
            .astype(np.float32)
        )
        for i in range(len(m)):
            row_off, nrows, _g = cells[i][c]
            src = int(starts[i]) * TILE
            if nrows > 0:
                out[row_off : row_off + nrows] = rows[src : src + nrows]
    return out


def kernel(A, B, batch_sizes, batch_offsets, batch_padded_offsets):
    global LAST_EXEC_NS
    from concourse.bass_utils import run_bass_kernel_spmd

    A = np.asarray(A, dtype=np.float32)
    B = np.asarray(B, dtype=np.float32)
    sizes = np.asarray(batch_sizes, dtype=np.int64)
    offsets = np.asarray(batch_offsets, dtype=np.int64)

    M = A.shape[0]
    cells, m = _schedule(sizes, offsets)
    T = int(sum(m))

    nc = _get_program(tuple(int(x) for x in m), MM_DTYPE, W_TILES)
    in_maps = _pack_inputs(A, B, cells, m, T)

    trace = bool(int(os.environ.get("BASS_GG_TRACE", "0")))
    res = run_bass_kernel_spmd(
        nc,
        in_maps,
        core_ids=list(range(NCORES)),
        trace=trace,
        tmpdir=os.environ.get("BASS_GG_TRACE_DIR") or None,
    )
    LAST_EXEC_NS = res.exec_time_ns

    return _unpack_outputs(res.results, cells, m, T, M)
